# revision 45
# baseline (speedup 1.0000x reference)
"""Trainium2 Bass kernel for nn_Attention_85186381348942.

2D self-attention block: x [2, 512, 64, 64], 8 heads x 64 dim, n = 4096 tokens.
  qkv = w_qkv @ x ; per head: S = (q*scale)^T k ; P = exp(S) (softmax without
  max-subtraction -- logits are small); out = (P/Z) @ v ; y = w_out @ out + b.

Sharding: 8 cores = (batch b in {0,1}) x (head-pair hp in {0..3}); each core
computes 2 heads of one batch and the partial output projection for its head
slice. Host sums the 4 partials per batch and adds bias.

Device-side layout notes (all matmul operands bf16, PSUM fp32):
 - scores are computed TRANSPOSED: S_T[j, i] = k^T q so that the softmax'd
   tile P [j, i] (j = key pos on partitions) feeds the PV matmul directly as
   the moving operand: out_T[d, i] = sum_j v'_T[j, d] P[j, i].
 - v' has a 65th "ones" column, so row 64 of out_T is the softmax denominator
   Z[i] -- numerator and denominator use identical bf16 P values, so their
   rounding cancels in the ratio.
 - Both heads' score matmuls are K=64 row-tiled pairs (head A at PE rows 0-63,
   head B at rows 64-127) running concurrently in the PE array; both write one
   [128, 1024] 2-bank PSUM tile so a single FD=1024 ACT exp covers both heads.
 - Softmax runs in the log2 domain: the host folds scale*log2(e)*128 into
   w_q, so scores arrive as s*128. EVERY step's exp is split BY HEAD across
   both engines: ACT exps head A's [128,512] tile (scale=ln2/128), and head
   B goes through EXP2_BITS_ANT, a custom fused 8-stage DVE op that builds
   the bf16 bit pattern of 2^s in one 1-elem/cycle pass (u = s128+16256;
   RN-to-multiple-of-128 via magic-add; |u-RN(u)| quadratic mantissa
   correction; int16 output convert = the bf16 bits). Each half (~570/~780
   ns) is under the per-step PE time (~1008ns), so exp latency hides
   entirely and each head's score psum bank is recycled by its OWN engine.
 - The per-head split also decouples the two pipelines:
   scoresA -> ACT exp -> PV_A and scoresB -> DVE exp -> PV_B share only the
   PE queue. Scores stay TWO steps ahead so a freed psum bank refills
   immediately.
 - The matmul shapes deliberately keep stationary columns <= moving columns
   (LDWEIGHTS loads P columns at 1.2GHz while the array streams moving
   columns at 2.4GHz; transposed-PV variants with 128-col stationary P
   chunks would be LDWEIGHTS-bound on real HW despite a lower cost-model
   estimate).
 - fp8 (incl. DoubleRow 2x matmuls) was evaluated and REJECTED: attention
   here is diffuse (out ~ mean of ~10^3 v's), so the output signal averages
   down exactly as fast as per-element weight noise -- relative error ==
   P-quantization rms (~2.8% for e5m2 P, measured on the real data), over
   the 2e-2 gate. Same argument kills fp8 q/k (score noise -> P noise).
   The kernel is at the bf16 PE roofline (~252us of matmul engine time).
 - To keep the engines fed: v^T chunks are built during ib0 two steps ahead
   of their PV (single ACT copy fills both heads' [.,2,65] slots of the
   interleaved vAB tile, preserving the ones columns), x block 0 lands
   per-o-chunk so the first qk matmuls start ~1.5us in, q i-blocks are
   prefetched one ib ahead (j=26..29, copies on ACT), each ib's epilogue
   (1/Z recip j=1, DMA-broadcast j=3, per-head normalize j=9/10 on DVE,
   projection halves j=11..25 odd on PE, [128,256] staging half-copies on
   ACT at j=14..28) is spread across the NEXT ib's steps so no single step
   overloads an engine.
"""

import numpy as np
import ml_dtypes

import concourse.bass as bass
import concourse.tile as tile
from concourse import bacc, mybir
from concourse.bass_utils import run_bass_kernel_spmd

BF16 = mybir.dt.bfloat16
F32 = mybir.dt.float32
I16 = mybir.dt.int16
AF = mybir.ActivationFunctionType

HEADS = 8
DIM_HEAD = 64
DIM = 512
N = 4096  # 64*64 tokens
N_CORES = 8
NB = 8  # number of 512-wide i-blocks
JB = 32  # number of 128-wide j-blocks
WI = 512  # i-block width

LN2 = float(np.log(2.0))
# Fused DVE exp2: scores arrive prescaled by 128 (u = s128 + 16256 IS the
# bf16 bit pattern of 2^s up to mantissa correction). One 8-stage custom op:
#   u = x + 16256; q = RN_128(u) via magic-add; a = |u - q|;
#   out_i16 = RN(u + a*(CB*a + CA))   [int16 bits == bf16 of 2^s]
# CA/CB minimax-fitted incl. final rounding: max rel err 0.54%.
EXP2_MAGIC = float(1.5 * 2**23 * 128)  # fp32 magic: round to multiple of 128
EXP2_CA = -0.327
EXP2_CB = 0.00232


_EXP2_OP = None


def _register_exp2_op():
    """Register the fused exp2-bits custom DVE op (idempotent)."""
    global _EXP2_OP
    if _EXP2_OP is not None:
        return _EXP2_OP
    import concourse.dve_ops as dve_ops_mod
    from concourse.dve_spec import (
        AluOp as SAluOp,
        Bin,
        C0,
        C1,
        C2,
        C3,
        Spec,
        Src0,
        _spill_c3_to_src1,
        lower,
    )
    from concourse.dve_uop import DveOpSpec

    name = "EXP2_BITS_ANT"
    for op in dve_ops_mod.OPS:
        if op.name == name:
            _EXP2_OP = op
            return op

    u = Src0 + C0
    q = (u + C1) - C1
    a = Bin(SAluOp.ABSOLUTE_DIFF, u, q)
    body = u + (a * C2 + C3) * a

    def _ref(in0, in1, s0, s1, imm2):
        uu = in0.astype(np.float32) + np.float32(s0)
        tt = (uu + np.float32(s1)).astype(np.float32)
        qq = (tt - np.float32(s1)).astype(np.float32)
        aa = np.abs(uu - qq).astype(np.float32)
        return (uu + (aa * np.float32(imm2) + in1.astype(np.float32)) * aa
                ).astype(np.float32)

    spec = Spec(body=_spill_c3_to_src1(body), reference=_ref)
    uops_sha = {
        ver: DveOpSpec(name=name, opcode=0, uops=lower(spec, ver=ver),
                       rd1_en=True).sha(ver)
        for ver in ("v3", "v4")
    }
    op = dve_ops_mod.DveOp(name, spec, subdim=False, uops_sha=uops_sha)
    dve_ops_mod.OPS.append(op)
    dve_ops_mod._SUB_OPCODE_FOR_NAME[name] = (
        max(dve_ops_mod._SUB_OPCODE_FOR_NAME.values()) + 1
    )
    _EXP2_OP = op
    return op


def build_program(repeats: int = 1, small_out: bool = False) -> bass.Bass:
    _register_exp2_op()
    nc = bacc.Bacc(None, target_bir_lowering=False, num_devices=N_CORES)

    x = nc.dram_tensor("x", [DIM, N], BF16, kind="ExternalInput")
    wqk = nc.dram_tensor("wqk", [DIM, 256], BF16, kind="ExternalInput")
    wv = nc.dram_tensor("wv", [DIM, 128], BF16, kind="ExternalInput")
    wo = nc.dram_tensor("wo", [128, DIM], BF16, kind="ExternalInput")
    if small_out:
        out = nc.dram_tensor("out", [4, 128, WI], F32, kind="ExternalOutput")
        out_r = out
    else:
        out = nc.dram_tensor("out", [DIM, N], F32, kind="ExternalOutput")
        out_r = out.rearrange("(o p) n -> o p n", p=128)

    x_r = x.rearrange("(o p) n -> p o n", p=128)
    wqk_r = wqk.rearrange("(o p) m -> p o m", p=128)
    wv_r = wv.rearrange("(o p) m -> p o m", p=128)

    with tile.TileContext(nc) as tc:
        with (
            tc.tile_pool(name="singles", bufs=1) as singles,
            tc.tile_pool(name="pp", bufs=4) as pp,  # P = exp(S) tiles
            tc.tile_pool(name="sbsm", bufs=2) as sbsm,  # small sbuf temps
            tc.tile_pool(name="stg", bufs=6) as stg,  # output staging
            tc.tile_pool(name="ps_s", bufs=2, space="PSUM") as ps_s,  # scores+misc
            tc.tile_pool(name="ps_o", bufs=1, space="PSUM") as ps_o,  # PV accum
            tc.tile_pool(name="dram", bufs=2, space="DRAM") as dram,
        ):
            # ---- load inputs: x on HWDGE, weights on SWDGE so the small
            # weight transfers don't serialize ahead of the big x transfer
            x_sb = singles.tile([128, 4, N], BF16)
            # n-blocks 0-2 land first (k/q block 0 + the early streamed
            # k-blocks); the rest as one large transfer, all pipelined with
            # the qk preamble and ib0's j-loop
            wqk_sb = singles.tile([128, 4, 256], BF16)
            # first-needed inputs fan out across DMA queues: SP.SEQ issue is
            # 650ns each SERIAL, so putting x chunk o0/o1 on the idle DVE/ACT
            # queues lets wqk + the first two x chunks all land by ~2.1us
            nc.sync.dma_start(wqk_sb[:], wqk_r)
            nc.scalar.dma_start(x_sb[:, 0, 0:WI], x_r[:, 0, 0:WI])
            nc.gpsimd.dma_start(x_sb[:, 1, 0:WI], x_r[:, 1, 0:WI])
            for o in (2, 3):
                nc.sync.dma_start(
                    x_sb[:, o, 0:WI], x_r[:, o, 0:WI])
            for nb0 in range(1, 4):
                nc.sync.dma_start(
                    x_sb[:, :, nb0 * WI:(nb0 + 1) * WI],
                    x_r[:, :, nb0 * WI:(nb0 + 1) * WI])
            nc.sync.dma_start(x_sb[:, :, 4 * WI:], x_r[:, :, 4 * WI:])
            wv_sb = singles.tile([128, 4, 128], BF16)
            nc.gpsimd.dma_start(wv_sb[:], wv_r)
            wo_sb = singles.tile([128, DIM], BF16)
            nc.gpsimd.dma_start(wo_sb[:], wo[:])

            q_sb = singles.tile([128, N], BF16)
            k_sb = singles.tile([128, N], BF16)
            # v'_T both heads interleaved: [j-part, chunk, head, d+1];
            # col 64 of each head slot = ones (softmax denominator row)
            # PE pstate warm-up fodder: memset FIRST so wide spin matmuls
            # can bridge the preamble DMA wait and carry the DVFS ramp
            warm_t = singles.tile([64, WI], BF16)
            nc.vector.memset(warm_t[:], 0.0)
            vAB = singles.tile([128, JB, 2, 65], BF16)
            nc.vector.memset(vAB[:, :, :, 64:65], 1.0)
            # ones column at partition 64 for the tail 1/Z PE-broadcast
            ones_bc = singles.tile([65, 64], F32)
            nc.vector.memset(ones_bc[:], 1.0)
            # per-partition CA constant for the fused DVE exp2 (Latch(Src1))
            ca_sb = singles.tile([128, 1], F32)
            nc.vector.memset(ca_sb[:], EXP2_CA)
            # dummy activation: pull the ~2.7us ACT table load into the x-DMA
            # window instead of paying it before the first real exp
            warm = singles.tile([1, 8], F32)
            nc.vector.memset(warm[:], 0.0)
            nc.scalar.activation(warm[:], warm[:], AF.Exp)
            def emit_qk_block(m, dst, nb, tag="sA", ps=None, o_only=None,
                              copy_eng="v"):
                """dst[:, nb] = wqk[:, m-chunk]^T @ x[:, nb-block] (q: m=0, k: m=1).
                With o_only set, emits just that contraction chunk (caller
                passes the same `ps` across the 4 chunks); returns ps."""
                if ps is None:
                    if tag in ("sA", "sB"):
                        ps = ps_s.tile([128, WI], F32, tag=tag, name="qk_ps")
                    else:
                        ps = ps_o.tile([128, WI], F32, tag=tag, name="qk_ps")
                chunks = range(4) if o_only is None else [o_only]
                for o in chunks:
                    nc.tensor.matmul(
                        ps[:],
                        lhsT=wqk_sb[:, o, m * 128:(m + 1) * 128],
                        rhs=x_sb[:, o, nb * WI:(nb + 1) * WI],
                        start=(o == 0),
                        stop=(o == 3),
                        skip_group_check=True,
                    )
                if o_only in (None, 3):
                    if copy_eng == "a":
                        nc.scalar.copy(dst[:, nb * WI:(nb + 1) * WI], ps[:])
                    else:
                        nc.vector.tensor_copy(dst[:, nb * WI:(nb + 1) * WI],
                                              ps[:])
                return ps

            def emit_vt_chunk(t):
                """v'_T chunk t for both heads: vT[n, d] = x^T @ wv.
                Runs during ib0 (parity 0), so the parity-1 out slots are
                free -- use them instead of loading the scores pool. ONE
                [128,128]-col ACT copy fills both head slots (the dst AP
                strides over the ones columns)."""
                ps = ps_o.tile([128, 128], F32, tag="outA1", name="vt_ps")
                for o in range(4):
                    nc.tensor.matmul(
                        ps[:],
                        lhsT=x_sb[:, o, t * 128:(t + 1) * 128],
                        rhs=wv_sb[:, o, :],
                        start=(o == 0),
                        stop=(o == 3),
                    )
                nc.scalar.copy(vAB[:, t, :, 0:64],
                               ps[:].rearrange("p (h d) -> p h d", h=2))

            # 6 wide spins: ~2.9us of continuous PE work from ~0.45us, so
            # the real matmuls (gated by x-o0 at ~2.5us) start near-ramped
            # instead of paying the half-speed mid pstate for ~3us
            warm_ps = ps_o.tile([64, WI], F32, tag="outB1", name="warm_ps")
            for _ in range(6):
                nc.tensor.matmul(warm_ps[:], lhsT=warm_t[:, 0:64],
                                 rhs=warm_t[:], start=True, stop=True,
                                 skip_group_check=True)

            for _rep in range(repeats):
                # minimum preamble: k-block 0 (covers j=0..3) + q-block 0;
                # k-blocks 1..7 are streamed inside ib0's j-loop
                kps = qps = None
                for o in range(4):
                    kps = emit_qk_block(1, k_sb, 0, tag="sA", ps=kps,
                                        o_only=o)
                    qps = emit_qk_block(0, q_sb, 0, tag="sB", ps=qps,
                                        o_only=o)
                # v^T chunks 0/1 up front; the loop builds j+2 at step j so
                # each chunk's ACT copy lands ~2 steps before PV reads it
                emit_vt_chunk(0)
                emit_vt_chunk(1)

                # ---- attention main loop ------------------------------------
                # epi[ib] = context dict for ib's deferred epilogue
                epi = {}

                def emit_epilogue_piece(ctx, piece):
                    ib = ctx["ib"]
                    i0 = ib * WI
                    par = ib % 2
                    if piece == 0:  # reciprocal of Z rows (DVE)
                        for h in (0, 1):
                            zrec = sbsm.tile([65, WI], F32, tag=f"zrec{h}",
                                             name="zrec")
                            nc.vector.reciprocal(
                                zrec[64:65, :], ctx["out_ps"][h][64:65, :])
                            ctx[f"zrec{h}"] = zrec
                    elif piece == 1:  # broadcast 1/Z across d
                        for h in (0, 1):
                            r_sb = sbsm.tile([64, WI], F32, tag=f"rsb{h}",
                                             name="r_sb")
                            if ib == NB - 1:
                                # tail: scores pool is idle -- PE broadcast is
                                # ~4us faster than the DRAM roundtrip
                                r_ps = ps_s.tile([64, WI], F32, tag="sA",
                                                 name="r_ps")
                                nc.tensor.matmul(
                                    r_ps[:], lhsT=ones_bc[64:65, :],
                                    rhs=ctx[f"zrec{h}"][64:65, :],
                                    start=True, stop=True)
                                nc.scalar.copy(r_sb[:], r_ps[:])
                            else:  # DMA engines only; no PE/PSUM use
                                zdr = dram.tile([1, WI], F32, tag=f"zdr{h}",
                                                name="zdr")
                                nc.sync.dma_start(zdr[:], ctx[f"zrec{h}"][64:65, :])
                                nc.sync.dma_start(r_sb[:],
                                                  zdr.to_broadcast([64, WI]))
                            ctx[f"rsb{h}"] = r_sb
                    elif piece == 2:  # normalize h=0 into outn_pair rows 0-63
                        if "outnp" not in ctx:
                            ctx["outnp"] = sbsm.tile([128, WI], BF16,
                                                     tag="outnp",
                                                     name="outn_pair")
                        nc.vector.tensor_mul(
                            ctx["outnp"][0:64, :], ctx["out_ps"][0][0:64, :],
                            ctx["rsb0"][:])
                    elif piece <= 6:  # pieces 3..6: projection chunks, K=128
                        # both heads' normalized outs stacked on partitions
                        # 0-63/64-127, so each chunk is ONE K=128 matmul
                        # (half the PE time of the per-head K=64 pair)
                        o = piece - 3
                        ctx[f"pr{o}"] = ps_o.tile(
                            [128, WI], F32, tag=f"out{'AB'[o % 2]}{1 - par}",
                            name="pr_ps")
                        nc.tensor.matmul(
                            ctx[f"pr{o}"][:],
                            lhsT=wo_sb[:, o * 128:(o + 1) * 128],
                            rhs=ctx["outnp"][:],
                            start=True, stop=True,
                        )
                    elif piece <= 18:  # pieces 11..18: stage+store half chunks
                        # ACT copies [128, 256] halves so no step's ACT load
                        # exceeds the PE cadence; DMA fires on the 2nd half.
                        # In the exposed tail DVE is idle after normalize, so
                        # the odd halves go there and staging drains 2x wide.
                        o, half = divmod(piece - 11, 2)
                        if half == 0:
                            ctx[f"st{o}"] = stg.tile([128, WI], F32, tag="st",
                                                     name="st")
                        st = ctx[f"st{o}"]
                        c0, c1 = half * 256, half * 256 + 256
                        if ib == NB - 1 and half == 1:
                            nc.vector.tensor_copy(st[:, c0:c1],
                                                  ctx[f"pr{o}"][:, c0:c1])
                        else:
                            nc.scalar.copy(st[:, c0:c1],
                                           ctx[f"pr{o}"][:, c0:c1])
                        if half == 1:
                            if small_out:
                                nc.sync.dma_start(out_r[o, :, :], st[:])
                            else:
                                nc.sync.dma_start(out_r[o, :, i0:i0 + WI],
                                                  st[:])
                    elif piece == 19:  # normalize h=1 (temp at partitions 0-63)
                        on = sbsm.tile([64, WI], BF16, tag="outn1", name="outn")
                        nc.vector.tensor_mul(
                            on[:], ctx["out_ps"][1][0:64, :], ctx["rsb1"][:])
                        ctx["outn1"] = on
                    else:  # piece 21: shift h1 to outn_pair rows 64-127 (DMA)
                        if "outnp" not in ctx:
                            ctx["outnp"] = sbsm.tile([128, WI], BF16,
                                                     tag="outnp",
                                                     name="outn_pair")
                        dq = nc.sync if ib == NB - 1 else nc.gpsimd
                        dq.dma_start(ctx["outnp"][64:128, :],
                                     ctx["outn1"][:])

                # epilogue(prev) piece schedule within the next i-block:
                # j=1 recip (DVE), j=3 DMA bcast, j=9 normalize (DVE, frees
                # prev's out psums), j=11..25 odd: projection halves (PE,
                # reuses them; one matmul per step), j=16..28 mod 4: staging
                # copies (ACT) on the exp-free steps
                PIECE_AT = {1: 0, 3: 1, 9: 2, 10: 19, 11: 21,
                            15: 3, 17: 4, 19: 5, 21: 6,
                            16: 11, 18: 12, 20: 13, 22: 14,
                            23: 15, 25: 16, 27: 17, 28: 18}

                def emit_scores_head(ib2, j2, h):
                    """One head's score tile: each head's psum bank is freed
                    by ITS exp engine alone (A: ACT, B: DVE), so the two
                    pipelines recycle independently."""
                    i0 = ib2 * WI
                    j0 = j2 * 128
                    t = ps_s.tile([128, WI], F32, tag="sA" if h == 0 else "sB",
                                  name="s_ps")
                    nc.tensor.matmul(
                        t[:],
                        lhsT=k_sb[64 * h:64 * h + 64, j0:j0 + 128],
                        rhs=q_sb[64 * h:64 * h + 64, i0:i0 + WI],
                        start=True, stop=True,
                    )
                    return t

                def emit_scores(ib2, j2):
                    return (emit_scores_head(ib2, j2, 0),
                            emit_scores_head(ib2, j2, 1))

                qps0 = qpsn = None
                s_tiles = {}
                outs = {}

                def alloc_outs(ib):
                    par = ib % 2
                    o_A = ps_o.tile([65, WI], F32, tag=f"outA{par}", name="out_A")
                    o_B = ps_o.tile([65, WI], F32, tag=f"outB{par}", name="out_B")
                    outs[ib] = (o_A, o_B)
                    epi[ib] = {"ib": ib, "out_ps": {0: o_A, 1: o_B}}

                def emit_pv(pv_ib, pv_j, pv_p, start, stop, heads=(0, 1)):
                    o_A, o_B = outs[pv_ib]
                    if 0 in heads:
                        nc.tensor.matmul(
                            o_A[:], lhsT=vAB[:, pv_j, 0, :], rhs=pv_p[:, 0:WI],
                            start=start, stop=stop, skip_group_check=True,
                        )
                    if 1 in heads:
                        nc.tensor.matmul(
                            o_B[:], lhsT=vAB[:, pv_j, 1, :],
                            rhs=pv_p[:, WI:2 * WI],
                            start=start, stop=stop, skip_group_check=True,
                        )

                def emit_exp(ib, j, s_tiles_pair, p_sb, all_act=False):
                    """exp split BY HEAD across both engines every step: ACT
                    exps head A's [128, 512] tile (bf16 out), the fused
                    EXP2_BITS_ANT DVE op does head B's (int16 bit-assembly of
                    the same 2^s values). Each half is well under the
                    per-step PE time, so exp latency hides behind PE and each
                    score bank recycles as soon as its OWN engine reads it.
                    all_act=True routes head B to ACT too (used on the step
                    whose DVE slot is taken by the 1/Z reciprocal)."""
                    sA, sB = s_tiles_pair
                    if all_act:
                        nc.scalar.activation(p_sb[:, WI:], sB[:],
                                             AF.Exp, scale=LN2 / 128.0)
                    else:
                        nc.vector._custom_dve(
                            _EXP2_OP,
                            out=p_sb[:, WI:].bitcast(I16),
                            in0=sB[:],
                            in1=ca_sb[:],
                            s0=16256.0,
                            s1=EXP2_MAGIC,
                            imm2=EXP2_CB,
                        )
                    nc.scalar.activation(p_sb[:, 0:WI], sA[:],
                                         AF.Exp, scale=LN2 / 128.0)

                alloc_outs(0)
                s_tiles[0] = emit_scores(0, 0)
                s_tiles[1] = emit_scores(0, 1)
                for gs in range(NB * JB):
                    ib, j = divmod(gs, JB)
                    par = ib % 2
                    out_A, out_B = outs[ib]
                    prev = epi.get(ib - 1)
                    # scores stay TWO global steps ahead: they then sit ahead
                    # of the PV pairs in the PE queue, so the moment exp(g)
                    # frees a psum slot the next scores pair issues first and
                    # the following exp starts ~0.4us earlier per beat
                    nxt = gs + 2 < NB * JB
                    if nxt:
                        nib, nj = divmod(gs + 2, JB)
                        if nj == 0:
                            alloc_outs(nib)
                    s_pair = s_tiles.pop(gs)
                    p_sb = pp.tile([128, 2 * WI], BF16, tag="p", name="p_sb")
                    emit_exp(ib, j, s_pair, p_sb)
                    # PE queue interleaves the two head pipelines:
                    # [scoresA', pvA, scoresB', pvB] -- when the slower DVE
                    # exp (head B) lags, head A's ready work is not stuck
                    # behind a blocked scoresB allocation
                    sA2 = emit_scores_head(nib, nj, 0) if nxt else None
                    emit_pv(ib, j, p_sb, j == 0, j == JB - 1, heads=(0,))
                    sB2 = emit_scores_head(nib, nj, 1) if nxt else None
                    if ib == 0 and j + 2 < JB:
                        # v^T built inline two chunks ahead of its PV
                        emit_vt_chunk(j + 2)
                    emit_pv(ib, j, p_sb, j == 0, j == JB - 1, heads=(1,))
                    if nxt:
                        s_tiles[gs + 2] = (sA2, sB2)
                    if prev is not None and j in PIECE_AT:
                        emit_epilogue_piece(prev, PIECE_AT[j])
                    if ib == 0:
                        # stream k-blocks 1..7: one 4-matmul burst every 4
                        # steps (block nb ready before scores j=4nb)
                        if j % 4 == 0 and j // 4 + 1 < NB:
                            emit_qk_block(1, k_sb, j // 4 + 1, tag="outB1",
                                          copy_eng="a")
                        # prefetch q block 1: one chunk per step, j=26..29
                        if 26 <= j <= 29:
                            qps0 = emit_qk_block(0, q_sb, 1, tag="outB1",
                                                 ps=qps0 if j > 26 else None,
                                                 o_only=j - 26, copy_eng="a")
                    elif ib + 1 < NB:
                        # prefetch next q i-block: one chunk per step,
                        # j=26..29, in the freed parity slot (done a step
                        # before the 2-ahead scores of the next ib need it)
                        if 26 <= j <= 29:
                            qpsn = emit_qk_block(0, q_sb, ib + 1,
                                                 tag=f"outA{1 - par}",
                                                 ps=qpsn if j > 26 else None,
                                                 o_only=j - 26, copy_eng="a")
                # exposed tail: stage+store each proj chunk right after its
                # second half so the output DMAs start draining immediately
                for piece in (0, 1, 19, 21, 2, 3, 11, 12, 4, 13, 14,
                              5, 15, 16, 6, 17, 18):
                    emit_epilogue_piece(epi[NB - 1], piece)

    nc.finalize()
    return nc


_PROGRAM_CACHE = {}


def _get_program(**kw) -> bass.Bass:
    key = tuple(sorted(kw.items()))
    if key not in _PROGRAM_CACHE:
        _PROGRAM_CACHE[key] = build_program(**kw)
    return _PROGRAM_CACHE[key]


def _prep_inputs(x, w_qkv, w_out):
    """Build the per-core input maps (all bf16 host-side casts)."""
    # q is pre-scaled by scale*log2(e): scores arrive in the log2 domain
    # (ACT exp uses scale=ln2; the DVE exp2 path uses them directly)
    scale = DIM_HEAD ** -0.5 * float(np.log2(np.e)) * 128.0
    xb = x.reshape(2, DIM, N)
    in_maps = []
    for core in range(N_CORES):
        b, hp = divmod(core, 4)
        r0 = hp * 128
        wq = w_qkv[r0:r0 + 128] * scale          # [128, 512]
        wk = w_qkv[DIM + r0:DIM + r0 + 128]      # [128, 512]
        wvr = w_qkv[2 * DIM + r0:2 * DIM + r0 + 128]
        wqk_c = np.concatenate([wq.T, wk.T], axis=1)   # [512, 256]
        wv_t = wvr.T                                   # [512, 128]
        wo_pair = np.concatenate(
            [w_out[:, r0:r0 + 64].T, w_out[:, r0 + 64:r0 + 128].T], axis=0
        )  # [128, 512]: rows 0-63 head A, 64-127 head B
        in_maps.append({
            "x": xb[b].astype(ml_dtypes.bfloat16),
            "wqk": wqk_c.astype(ml_dtypes.bfloat16),
            "wv": wv_t.astype(ml_dtypes.bfloat16),
            "wo": wo_pair.astype(ml_dtypes.bfloat16),
        })
    return in_maps


def _run(nc, in_maps):
    try:
        return run_bass_kernel_spmd(nc, in_maps, core_ids=list(range(N_CORES)))
    except Exception:
        # one retry: a previously-wedged device surfaces as a transient
        # NRT_EXEC_UNIT_UNRECOVERABLE on the first execution
        return run_bass_kernel_spmd(nc, in_maps, core_ids=list(range(N_CORES)))


def kernel(x, w_qkv, w_out, b_out):
    nc = _get_program()
    in_maps = _prep_inputs(np.asarray(x), np.asarray(w_qkv), np.asarray(w_out))
    res = _run(nc, in_maps)
    partials = np.stack([r["out"] for r in res.results])  # [8, 512, 4096]
    y = partials.reshape(2, 4, DIM, N).sum(axis=1)
    y += np.asarray(b_out)[None, :, None]
    return y.reshape(2, DIM, 64, 64).astype(np.float32)



# revision 51
# speedup vs baseline: 1.0030x; 1.0030x over previous
"""Trainium2 Bass kernel for nn_Attention_85186381348942.

2D self-attention block: x [2, 512, 64, 64], 8 heads x 64 dim, n = 4096 tokens.
  qkv = w_qkv @ x ; per head: S = (q*scale)^T k ; P = exp(S) (softmax without
  max-subtraction -- logits are small); out = (P/Z) @ v ; y = w_out @ out + b.

Sharding: 8 cores = (batch b in {0,1}) x (head-pair hp in {0..3}); each core
computes 2 heads of one batch and the partial output projection for its head
slice. Host sums the 4 partials per batch and adds bias.

Device-side layout notes (all matmul operands bf16, PSUM fp32):
 - scores are computed TRANSPOSED: S_T[j, i] = k^T q so that the softmax'd
   tile P [j, i] (j = key pos on partitions) feeds the PV matmul directly as
   the moving operand: out_T[d, i] = sum_j v'_T[j, d] P[j, i].
 - v' has a 65th "ones" column, so row 64 of out_T is the softmax denominator
   Z[i] -- numerator and denominator use identical bf16 P values, so their
   rounding cancels in the ratio.
 - Both heads' score matmuls are K=64 row-tiled pairs (head A at PE rows 0-63,
   head B at rows 64-127) running concurrently in the PE array; both write one
   [128, 1024] 2-bank PSUM tile so a single FD=1024 ACT exp covers both heads.
 - Softmax runs in the log2 domain: the host folds scale*log2(e)*128 into
   w_q, so scores arrive as s*128. EVERY step's exp is split BY HEAD across
   both engines: ACT exps head A's [128,512] tile (scale=ln2/128), and head
   B goes through EXP2_BITS_ANT, a custom fused 8-stage DVE op that builds
   the bf16 bit pattern of 2^s in one 1-elem/cycle pass (u = s128+16256;
   RN-to-multiple-of-128 via magic-add; |u-RN(u)| quadratic mantissa
   correction; int16 output convert = the bf16 bits). Each half (~570/~780
   ns) is under the per-step PE time (~1008ns), so exp latency hides
   entirely and each head's score psum bank is recycled by its OWN engine.
 - The per-head split also decouples the two pipelines:
   scoresA -> ACT exp -> PV_A and scoresB -> DVE exp -> PV_B share only the
   PE queue. Scores stay TWO steps ahead so a freed psum bank refills
   immediately.
 - The matmul shapes deliberately keep stationary columns <= moving columns
   (LDWEIGHTS loads P columns at 1.2GHz while the array streams moving
   columns at 2.4GHz; transposed-PV variants with 128-col stationary P
   chunks would be LDWEIGHTS-bound on real HW despite a lower cost-model
   estimate).
 - fp8 (incl. DoubleRow 2x matmuls) was evaluated and REJECTED: attention
   here is diffuse (out ~ mean of ~10^3 v's), so the output signal averages
   down exactly as fast as per-element weight noise -- relative error ==
   P-quantization rms (~2.8% for e5m2 P, measured on the real data), over
   the 2e-2 gate. Same argument kills fp8 q/k (score noise -> P noise).
   The kernel is at the bf16 PE roofline (~252us of matmul engine time).
 - To keep the engines fed: v^T chunks are built during ib0 two steps ahead
   of their PV (single ACT copy fills both heads' [.,2,65] slots of the
   interleaved vAB tile, preserving the ones columns), x block 0 lands
   per-o-chunk so the first qk matmuls start ~1.5us in, q i-blocks are
   prefetched one ib ahead (j=26..29, copies on ACT), each ib's epilogue
   (1/Z recip j=1, DMA-broadcast j=3, per-head normalize j=9/10 on DVE,
   projection halves j=11..25 odd on PE, [128,256] staging half-copies on
   ACT at j=14..28) is spread across the NEXT ib's steps so no single step
   overloads an engine.
"""

import numpy as np
import ml_dtypes

import concourse.bass as bass
import concourse.tile as tile
from concourse import bacc, mybir
from concourse.bass_utils import run_bass_kernel_spmd

BF16 = mybir.dt.bfloat16
F32 = mybir.dt.float32
I16 = mybir.dt.int16
AF = mybir.ActivationFunctionType

HEADS = 8
DIM_HEAD = 64
DIM = 512
N = 4096  # 64*64 tokens
N_CORES = 8
NB = 8  # number of 512-wide i-blocks
JB = 32  # number of 128-wide j-blocks
WI = 512  # i-block width

LN2 = float(np.log(2.0))
# Fused DVE exp2: scores arrive prescaled by 128 (u = s128 + 16256 IS the
# bf16 bit pattern of 2^s up to mantissa correction). One 8-stage custom op:
#   u = x + 16256; q = RN_128(u) via magic-add; a = |u - q|;
#   out_i16 = RN(u + a*(CB*a + CA))   [int16 bits == bf16 of 2^s]
# CA/CB minimax-fitted incl. final rounding: max rel err 0.54%.
EXP2_MAGIC = float(1.5 * 2**23 * 128)  # fp32 magic: round to multiple of 128
EXP2_CA = -0.327
EXP2_CB = 0.00232


_EXP2_OP = None


def _register_exp2_op():
    """Register the fused exp2-bits custom DVE op (idempotent)."""
    global _EXP2_OP
    if _EXP2_OP is not None:
        return _EXP2_OP
    import concourse.dve_ops as dve_ops_mod
    from concourse.dve_spec import (
        AluOp as SAluOp,
        Bin,
        C0,
        C1,
        C2,
        C3,
        Spec,
        Src0,
        _spill_c3_to_src1,
        lower,
    )
    from concourse.dve_uop import DveOpSpec

    name = "EXP2_BITS_ANT"
    for op in dve_ops_mod.OPS:
        if op.name == name:
            _EXP2_OP = op
            return op

    u = Src0 + C0
    q = (u + C1) - C1
    a = Bin(SAluOp.ABSOLUTE_DIFF, u, q)
    body = u + (a * C2 + C3) * a

    def _ref(in0, in1, s0, s1, imm2):
        uu = in0.astype(np.float32) + np.float32(s0)
        tt = (uu + np.float32(s1)).astype(np.float32)
        qq = (tt - np.float32(s1)).astype(np.float32)
        aa = np.abs(uu - qq).astype(np.float32)
        return (uu + (aa * np.float32(imm2) + in1.astype(np.float32)) * aa
                ).astype(np.float32)

    spec = Spec(body=_spill_c3_to_src1(body), reference=_ref)
    uops_sha = {
        ver: DveOpSpec(name=name, opcode=0, uops=lower(spec, ver=ver),
                       rd1_en=True).sha(ver)
        for ver in ("v3", "v4")
    }
    op = dve_ops_mod.DveOp(name, spec, subdim=False, uops_sha=uops_sha)
    dve_ops_mod.OPS.append(op)
    dve_ops_mod._SUB_OPCODE_FOR_NAME[name] = (
        max(dve_ops_mod._SUB_OPCODE_FOR_NAME.values()) + 1
    )
    _EXP2_OP = op
    return op


def build_program(repeats: int = 1, small_out: bool = False) -> bass.Bass:
    _register_exp2_op()
    nc = bacc.Bacc(None, target_bir_lowering=False, num_devices=N_CORES)

    x = nc.dram_tensor("x", [DIM, N], BF16, kind="ExternalInput")
    wqk = nc.dram_tensor("wqk", [DIM, 256], BF16, kind="ExternalInput")
    wv = nc.dram_tensor("wv", [DIM, 128], BF16, kind="ExternalInput")
    wo = nc.dram_tensor("wo", [128, DIM], BF16, kind="ExternalInput")
    if small_out:
        out = nc.dram_tensor("out", [4, 128, WI], F32, kind="ExternalOutput")
        out_r = out
    else:
        out = nc.dram_tensor("out", [DIM, N], F32, kind="ExternalOutput")
        out_r = out.rearrange("(o p) n -> o p n", p=128)

    x_r = x.rearrange("(o p) n -> p o n", p=128)
    wqk_r = wqk.rearrange("(o p) m -> p o m", p=128)
    wv_r = wv.rearrange("(o p) m -> p o m", p=128)

    with tile.TileContext(nc) as tc:
        with (
            tc.tile_pool(name="singles", bufs=1) as singles,
            tc.tile_pool(name="pp", bufs=4) as pp,  # P = exp(S) tiles
            tc.tile_pool(name="sbsm", bufs=2) as sbsm,  # small sbuf temps
            tc.tile_pool(name="stg", bufs=6) as stg,  # output staging
            tc.tile_pool(name="ps_s", bufs=2, space="PSUM") as ps_s,  # scores+misc
            tc.tile_pool(name="ps_o", bufs=1, space="PSUM") as ps_o,  # PV accum
            tc.tile_pool(name="dram", bufs=2, space="DRAM") as dram,
        ):
            # ---- load inputs: x on HWDGE, weights on SWDGE so the small
            # weight transfers don't serialize ahead of the big x transfer
            x_sb = singles.tile([128, 4, N], BF16)
            # n-blocks 0-2 land first (k/q block 0 + the early streamed
            # k-blocks); the rest as one large transfer, all pipelined with
            # the qk preamble and ib0's j-loop
            wqk_sb = singles.tile([128, 4, 256], BF16)
            # first-needed inputs fan out across DMA queues: SP.SEQ issue is
            # 650ns each SERIAL, so putting x chunk o0/o1 on the idle DVE/ACT
            # queues lets wqk + the first two x chunks all land by ~2.1us
            nc.sync.dma_start(wqk_sb[:], wqk_r)
            nc.scalar.dma_start(x_sb[:, 0, 0:WI], x_r[:, 0, 0:WI])
            nc.gpsimd.dma_start(x_sb[:, 1, 0:WI], x_r[:, 1, 0:WI])
            for o in (2, 3):
                nc.sync.dma_start(
                    x_sb[:, o, 0:WI], x_r[:, o, 0:WI])
            for nb0 in range(1, 4):
                nc.sync.dma_start(
                    x_sb[:, :, nb0 * WI:(nb0 + 1) * WI],
                    x_r[:, :, nb0 * WI:(nb0 + 1) * WI])
            nc.sync.dma_start(x_sb[:, :, 4 * WI:], x_r[:, :, 4 * WI:])
            wv_sb = singles.tile([128, 4, 128], BF16)
            nc.gpsimd.dma_start(wv_sb[:], wv_r)
            wo_sb = singles.tile([128, DIM], BF16)
            nc.gpsimd.dma_start(wo_sb[:], wo[:])

            q_sb = singles.tile([128, N], BF16)
            k_sb = singles.tile([128, N], BF16)
            # v'_T both heads interleaved: [j-part, chunk, head, d+1];
            # col 64 of each head slot = ones (softmax denominator row)
            # PE pstate warm-up fodder: memset FIRST so wide spin matmuls
            # can bridge the preamble DMA wait and carry the DVFS ramp
            warm_t = singles.tile([64, WI], BF16)
            nc.vector.memset(warm_t[:], 0.0)
            vAB = singles.tile([128, JB, 2, 65], BF16)
            nc.vector.memset(vAB[:, :, :, 64:65], 1.0)
            # ones column at partition 64 for the tail 1/Z PE-broadcast
            ones_bc = singles.tile([65, 64], F32)
            nc.vector.memset(ones_bc[:], 1.0)
            # per-partition CA constant for the fused DVE exp2 (Latch(Src1))
            ca_sb = singles.tile([128, 1], F32)
            nc.vector.memset(ca_sb[:], EXP2_CA)
            # dummy activation: pull the ~2.7us ACT table load into the x-DMA
            # window instead of paying it before the first real exp
            warm = singles.tile([1, 8], F32)
            nc.vector.memset(warm[:], 0.0)
            nc.scalar.activation(warm[:], warm[:], AF.Exp)
            def emit_qk_block(m, dst, nb, tag="sA", ps=None, o_only=None,
                              copy_eng="v"):
                """dst[:, nb] = wqk[:, m-chunk]^T @ x[:, nb-block] (q: m=0, k: m=1).
                With o_only set, emits just that contraction chunk (caller
                passes the same `ps` across the 4 chunks); returns ps."""
                if ps is None:
                    if tag in ("sA", "sB"):
                        ps = ps_s.tile([128, WI], F32, tag=tag, name="qk_ps")
                    else:
                        ps = ps_o.tile([128, WI], F32, tag=tag, name="qk_ps")
                chunks = range(4) if o_only is None else [o_only]
                for o in chunks:
                    nc.tensor.matmul(
                        ps[:],
                        lhsT=wqk_sb[:, o, m * 128:(m + 1) * 128],
                        rhs=x_sb[:, o, nb * WI:(nb + 1) * WI],
                        start=(o == 0),
                        stop=(o == 3),
                        skip_group_check=True,
                    )
                if o_only in (None, 3):
                    if copy_eng == "a":
                        nc.scalar.copy(dst[:, nb * WI:(nb + 1) * WI], ps[:])
                    else:
                        nc.vector.tensor_copy(dst[:, nb * WI:(nb + 1) * WI],
                                              ps[:])
                return ps

            def emit_vt_chunk(t):
                """v'_T chunk t for both heads: vT[n, d] = x^T @ wv.
                Runs during ib0 (parity 0), so the parity-1 out slots are
                free -- use them instead of loading the scores pool. ONE
                [128,128]-col ACT copy fills both head slots (the dst AP
                strides over the ones columns)."""
                ps = ps_o.tile([128, 128], F32, tag="outA1", name="vt_ps")
                for o in range(4):
                    nc.tensor.matmul(
                        ps[:],
                        lhsT=x_sb[:, o, t * 128:(t + 1) * 128],
                        rhs=wv_sb[:, o, :],
                        start=(o == 0),
                        stop=(o == 3),
                    )
                nc.scalar.copy(vAB[:, t, :, 0:64],
                               ps[:].rearrange("p (h d) -> p h d", h=2))

            # 6 wide spins: ~2.9us of continuous PE work from ~0.45us, so
            # the real matmuls (gated by x-o0 at ~2.5us) start near-ramped
            # instead of paying the half-speed mid pstate for ~3us
            warm_ps = ps_o.tile([64, WI], F32, tag="outB1", name="warm_ps")
            for _ in range(6):
                nc.tensor.matmul(warm_ps[:], lhsT=warm_t[:, 0:64],
                                 rhs=warm_t[:], start=True, stop=True,
                                 skip_group_check=True)

            for _rep in range(repeats):
                # minimum preamble: k-block 0 (covers j=0..3) + q-block 0;
                # k-blocks 1..7 are streamed inside ib0's j-loop
                kps = qps = None
                for o in range(4):
                    kps = emit_qk_block(1, k_sb, 0, tag="sA", ps=kps,
                                        o_only=o)
                    qps = emit_qk_block(0, q_sb, 0, tag="sB", ps=qps,
                                        o_only=o)
                # v^T chunks 0/1 up front; the loop builds j+2 at step j so
                # each chunk's ACT copy lands ~2 steps before PV reads it
                emit_vt_chunk(0)
                emit_vt_chunk(1)

                # ---- attention main loop ------------------------------------
                # epi[ib] = context dict for ib's deferred epilogue
                epi = {}

                def emit_epilogue_piece(ctx, piece):
                    ib = ctx["ib"]
                    i0 = ib * WI
                    par = ib % 2
                    if piece == 0:  # reciprocal of Z rows (DVE)
                        for h in (0, 1):
                            zrec = sbsm.tile([65, WI], F32, tag=f"zrec{h}",
                                             name="zrec")
                            nc.vector.reciprocal(
                                zrec[64:65, :], ctx["out_ps"][h][64:65, :])
                            ctx[f"zrec{h}"] = zrec
                    elif piece == 1:  # broadcast 1/Z across d
                        for h in (0, 1):
                            r_sb = sbsm.tile([64, WI], F32, tag=f"rsb{h}",
                                             name="r_sb")
                            if ib == NB - 1:
                                # tail: scores pool is idle -- PE broadcast is
                                # ~4us faster than the DRAM roundtrip
                                r_ps = ps_s.tile([64, WI], F32, tag="sA",
                                                 name="r_ps")
                                nc.tensor.matmul(
                                    r_ps[:], lhsT=ones_bc[64:65, :],
                                    rhs=ctx[f"zrec{h}"][64:65, :],
                                    start=True, stop=True)
                                nc.scalar.copy(r_sb[:], r_ps[:])
                            else:  # DMA engines only; no PE/PSUM use
                                zdr = dram.tile([1, WI], F32, tag=f"zdr{h}",
                                                name="zdr")
                                nc.sync.dma_start(zdr[:], ctx[f"zrec{h}"][64:65, :])
                                nc.sync.dma_start(r_sb[:],
                                                  zdr.to_broadcast([64, WI]))
                            ctx[f"rsb{h}"] = r_sb
                    elif piece == 2:  # normalize h=0 into outn_pair rows 0-63
                        if "outnp" not in ctx:
                            ctx["outnp"] = sbsm.tile([128, WI], BF16,
                                                     tag="outnp",
                                                     name="outn_pair")
                        nc.vector.tensor_mul(
                            ctx["outnp"][0:64, :], ctx["out_ps"][0][0:64, :],
                            ctx["rsb0"][:])
                    elif piece <= 6:  # pieces 3..6: projection chunks, K=128
                        # both heads' normalized outs stacked on partitions
                        # 0-63/64-127, so each chunk is ONE K=128 matmul
                        # (half the PE time of the per-head K=64 pair)
                        o = piece - 3
                        ctx[f"pr{o}"] = ps_o.tile(
                            [128, WI], F32, tag=f"out{'AB'[o % 2]}{1 - par}",
                            name="pr_ps")
                        nc.tensor.matmul(
                            ctx[f"pr{o}"][:],
                            lhsT=wo_sb[:, o * 128:(o + 1) * 128],
                            rhs=ctx["outnp"][:],
                            start=True, stop=True,
                        )
                    elif piece <= 18:  # pieces 11..18: stage+store half chunks
                        # ACT copies [128, 256] halves so no step's ACT load
                        # exceeds the PE cadence; DMA fires on the 2nd half.
                        # In the exposed tail DVE is idle after normalize, so
                        # the odd halves go there and staging drains 2x wide.
                        o, half = divmod(piece - 11, 2)
                        if half == 0:
                            ctx[f"st{o}"] = stg.tile([128, WI], F32, tag="st",
                                                     name="st")
                        st = ctx[f"st{o}"]
                        c0, c1 = half * 256, half * 256 + 256
                        if ib == NB - 1 and half == 1:
                            nc.vector.tensor_copy(st[:, c0:c1],
                                                  ctx[f"pr{o}"][:, c0:c1])
                        else:
                            nc.scalar.copy(st[:, c0:c1],
                                           ctx[f"pr{o}"][:, c0:c1])
                        if half == 1:
                            if small_out:
                                nc.sync.dma_start(out_r[o, :, :], st[:])
                            else:
                                nc.sync.dma_start(out_r[o, :, i0:i0 + WI],
                                                  st[:])
                    elif piece == 19:  # normalize h=1 (temp at partitions 0-63)
                        on = sbsm.tile([64, WI], BF16, tag="outn1", name="outn")
                        nc.vector.tensor_mul(
                            on[:], ctx["out_ps"][1][0:64, :], ctx["rsb1"][:])
                        ctx["outn1"] = on
                    else:  # piece 21: shift h1 to outn_pair rows 64-127 (DMA)
                        if "outnp" not in ctx:
                            ctx["outnp"] = sbsm.tile([128, WI], BF16,
                                                     tag="outnp",
                                                     name="outn_pair")
                        dq = nc.sync if ib == NB - 1 else nc.gpsimd
                        dq.dma_start(ctx["outnp"][64:128, :],
                                     ctx["outn1"][:])

                # epilogue(prev) piece schedule within the next i-block:
                # j=1 recip (DVE), j=3 DMA bcast, j=9 normalize (DVE, frees
                # prev's out psums), j=11..25 odd: projection halves (PE,
                # reuses them; one matmul per step), j=16..28 mod 4: staging
                # copies (ACT) on the exp-free steps
                PIECE_AT = {1: 0, 3: 1, 9: 2, 10: 19, 11: 21,
                            15: 3, 17: 4, 19: 5, 21: 6,
                            16: 11, 18: 12, 20: 13, 22: 14,
                            23: 15, 25: 16, 27: 17, 28: 18}

                def emit_scores_head(ib2, j2, h):
                    """One head's score tile: each head's psum bank is freed
                    by ITS exp engine alone (A: ACT, B: DVE), so the two
                    pipelines recycle independently."""
                    i0 = ib2 * WI
                    j0 = j2 * 128
                    t = ps_s.tile([128, WI], F32, tag="sA" if h == 0 else "sB",
                                  name="s_ps")
                    nc.tensor.matmul(
                        t[:],
                        lhsT=k_sb[64 * h:64 * h + 64, j0:j0 + 128],
                        rhs=q_sb[64 * h:64 * h + 64, i0:i0 + WI],
                        start=True, stop=True,
                    )
                    return t

                def emit_scores(ib2, j2):
                    return (emit_scores_head(ib2, j2, 0),
                            emit_scores_head(ib2, j2, 1))

                qps0 = qpsn = None
                s_tiles = {}
                outs = {}

                def alloc_outs(ib):
                    par = ib % 2
                    o_A = ps_o.tile([65, WI], F32, tag=f"outA{par}", name="out_A")
                    o_B = ps_o.tile([65, WI], F32, tag=f"outB{par}", name="out_B")
                    outs[ib] = (o_A, o_B)
                    epi[ib] = {"ib": ib, "out_ps": {0: o_A, 1: o_B}}

                def emit_pv(pv_ib, pv_j, pv_p, start, stop, heads=(0, 1)):
                    o_A, o_B = outs[pv_ib]
                    if 0 in heads:
                        nc.tensor.matmul(
                            o_A[:], lhsT=vAB[:, pv_j, 0, :], rhs=pv_p[:, 0:WI],
                            start=start, stop=stop, skip_group_check=True,
                        )
                    if 1 in heads:
                        nc.tensor.matmul(
                            o_B[:], lhsT=vAB[:, pv_j, 1, :],
                            rhs=pv_p[:, WI:2 * WI],
                            start=start, stop=stop, skip_group_check=True,
                        )

                def emit_exp(ib, j, s_tiles_pair, p_sb, all_act=False):
                    """exp split BY HEAD across both engines every step: ACT
                    exps head A's [128, 512] tile (bf16 out), the fused
                    EXP2_BITS_ANT DVE op does head B's (int16 bit-assembly of
                    the same 2^s values). Each half is well under the
                    per-step PE time, so exp latency hides behind PE and each
                    score bank recycles as soon as its OWN engine reads it.
                    all_act=True routes head B to ACT too (used on the step
                    whose DVE slot is taken by the 1/Z reciprocal)."""
                    sA, sB = s_tiles_pair
                    if all_act:
                        nc.scalar.activation(p_sb[:, WI:], sB[:],
                                             AF.Exp, scale=LN2 / 128.0)
                    else:
                        nc.vector._custom_dve(
                            _EXP2_OP,
                            out=p_sb[:, WI:].bitcast(I16),
                            in0=sB[:],
                            in1=ca_sb[:],
                            s0=16256.0,
                            s1=EXP2_MAGIC,
                            imm2=EXP2_CB,
                        )
                    nc.scalar.activation(p_sb[:, 0:WI], sA[:],
                                         AF.Exp, scale=LN2 / 128.0)

                alloc_outs(0)
                s_tiles[0] = emit_scores(0, 0)
                s_tiles[1] = emit_scores(0, 1)
                for gs in range(NB * JB):
                    ib, j = divmod(gs, JB)
                    par = ib % 2
                    out_A, out_B = outs[ib]
                    prev = epi.get(ib - 1)
                    # scores stay TWO global steps ahead: they then sit ahead
                    # of the PV pairs in the PE queue, so the moment exp(g)
                    # frees a psum slot the next scores pair issues first and
                    # the following exp starts ~0.4us earlier per beat
                    nxt = gs + 2 < NB * JB
                    if nxt:
                        nib, nj = divmod(gs + 2, JB)
                        if nj == 0:
                            alloc_outs(nib)
                    s_pair = s_tiles.pop(gs)
                    p_sb = pp.tile([128, 2 * WI], BF16, tag="p", name="p_sb")
                    emit_exp(ib, j, s_pair, p_sb)
                    # PE queue interleaves the two head pipelines:
                    # [scoresA', pvA, scoresB', pvB] -- when the slower DVE
                    # exp (head B) lags, head A's ready work is not stuck
                    # behind a blocked scoresB allocation
                    sA2 = emit_scores_head(nib, nj, 0) if nxt else None
                    emit_pv(ib, j, p_sb, j == 0, j == JB - 1, heads=(0,))
                    sB2 = emit_scores_head(nib, nj, 1) if nxt else None
                    if ib == 0 and j + 2 < JB:
                        # v^T built inline two chunks ahead of its PV
                        emit_vt_chunk(j + 2)
                    emit_pv(ib, j, p_sb, j == 0, j == JB - 1, heads=(1,))
                    if nxt:
                        s_tiles[gs + 2] = (sA2, sB2)
                    if prev is not None and j in PIECE_AT:
                        emit_epilogue_piece(prev, PIECE_AT[j])
                    if ib == 0:
                        # stream k-blocks 1..7: one 4-matmul burst every 4
                        # steps (block nb ready before scores j=4nb)
                        if j % 4 == 0 and j // 4 + 1 < NB:
                            emit_qk_block(1, k_sb, j // 4 + 1, tag="outB1",
                                          copy_eng="a")
                        # prefetch q block 1: one chunk per step, j=26..29
                        if 26 <= j <= 29:
                            qps0 = emit_qk_block(0, q_sb, 1, tag="outB1",
                                                 ps=qps0 if j > 26 else None,
                                                 o_only=j - 26, copy_eng="a")
                    elif ib + 1 < NB:
                        # prefetch next q i-block: one chunk per step,
                        # j=26..29, in the freed parity slot (done a step
                        # before the 2-ahead scores of the next ib need it)
                        if 26 <= j <= 29:
                            qpsn = emit_qk_block(0, q_sb, ib + 1,
                                                 tag=f"outA{1 - par}",
                                                 ps=qpsn if j > 26 else None,
                                                 o_only=j - 26, copy_eng="a")
                # exposed tail: stage+store each proj chunk right after its
                # second half so the output DMAs start draining immediately
                for piece in (0, 1, 19, 21, 2, 3, 11, 12, 4, 13, 14,
                              5, 15, 16, 6, 17, 18):
                    emit_epilogue_piece(epi[NB - 1], piece)

    nc.finalize()
    return nc


_PROGRAM_CACHE = {}


def _get_program(**kw) -> bass.Bass:
    key = tuple(sorted(kw.items()))
    if key not in _PROGRAM_CACHE:
        _PROGRAM_CACHE[key] = build_program(**kw)
    return _PROGRAM_CACHE[key]


def _prep_inputs(x, w_qkv, w_out):
    """Build the per-core input maps (all bf16 host-side casts)."""
    # q is pre-scaled by scale*log2(e): scores arrive in the log2 domain
    # (ACT exp uses scale=ln2; the DVE exp2 path uses them directly)
    scale = DIM_HEAD ** -0.5 * float(np.log2(np.e)) * 128.0
    xb = x.reshape(2, DIM, N)
    in_maps = []
    for core in range(N_CORES):
        b, hp = divmod(core, 4)
        r0 = hp * 128
        wq = w_qkv[r0:r0 + 128] * scale          # [128, 512]
        wk = w_qkv[DIM + r0:DIM + r0 + 128]      # [128, 512]
        wvr = w_qkv[2 * DIM + r0:2 * DIM + r0 + 128]
        wqk_c = np.concatenate([wq.T, wk.T], axis=1)   # [512, 256]
        wv_t = wvr.T                                   # [512, 128]
        wo_pair = np.concatenate(
            [w_out[:, r0:r0 + 64].T, w_out[:, r0 + 64:r0 + 128].T], axis=0
        )  # [128, 512]: rows 0-63 head A, 64-127 head B
        in_maps.append({
            "x": xb[b].astype(ml_dtypes.bfloat16),
            "wqk": wqk_c.astype(ml_dtypes.bfloat16),
            "wv": wv_t.astype(ml_dtypes.bfloat16),
            "wo": wo_pair.astype(ml_dtypes.bfloat16),
        })
    return in_maps


def _run(nc, in_maps):
    try:
        return run_bass_kernel_spmd(nc, in_maps, core_ids=list(range(N_CORES)))
    except Exception:
        # one retry: a previously-wedged device surfaces as a transient
        # NRT_EXEC_UNIT_UNRECOVERABLE on the first execution
        return run_bass_kernel_spmd(nc, in_maps, core_ids=list(range(N_CORES)))


def kernel(x, w_qkv, w_out, b_out):
    nc = _get_program()
    in_maps = _prep_inputs(np.asarray(x), np.asarray(w_qkv), np.asarray(w_out))
    res = _run(nc, in_maps)
    partials = np.stack([r["out"] for r in res.results])  # [8, 512, 4096]
    y = partials.reshape(2, 4, DIM, N).sum(axis=1)
    y += np.asarray(b_out)[None, :, None]
    return y.reshape(2, DIM, 64, 64).astype(np.float32)

